# revision 4
# baseline (speedup 1.0000x reference)
"""Trainium2 Bass kernel for 16-head causal MHA (B=2, T=4096, D=1024).

Sharding: 8 cores = 2 batches x 4 head-groups (4 heads each).
Each core computes Q/K/V projections for its 256 cols of Wq/Wk/Wv,
streaming causal attention for its 4 heads, and a partial output
projection against its 256 rows of Wo.  Host sums the 4 partials per
batch and adds the output bias.

Device layouts (everything "transposed", T on the free axis):
  xT   [1024, 4096] bf16  (x[b].T)
  Qt/Kt as 2 SBUF tensors [128, 4096] packing 2 heads (64 rows each)
  V    stored per 128-row T-chunk as [128, 4*65(+pad)] with a ones
       column appended per head ([V|1] trick: PV matmul row 64 = sum(P))
  S^T  computed per (head, 512-query-block, 128-key-chunk) as
       matmul(lhsT=Kt chunk [64,128], rhs=Qt block [64,512]) -> PSUM
  P    = exp(S/8) on ScalarE (PSUM fp32 -> SBUF bf16), causal masking of
       diagonal tiles via gpsimd affine_select
  ctx^T accumulated in PSUM ([V|1] stationary, P^T moving), normalized
       by 1/l with a gpsimd partition-broadcast of the row-64 sums
  out  o^T [1024, 4096] fp32 = Wo_slice^T @ ctx^T, host transposes+sums
"""

import math

import numpy as np

B, T, D = 2, 4096, 1024
H, HD = 16, 64
NCORES = 8
HPC = 4               # heads per core
DQ = HPC * HD         # 256 per-core projection width
P = 128
TQB = 512             # query block
NQB = T // TQB        # 8
NDC = D // P          # 8 contraction chunks for projections
NTC = T // P          # 32 key/T chunks
VST = 328             # per-T-chunk V stride: 4 heads * 65 + 63 pad

_NC_CACHE = {}


def _build_nc():
    import concourse.mybir as mybir
    from concourse import bacc
    from concourse.tile import TileContext

    dt = mybir.dt
    bf = dt.bfloat16
    f32 = dt.float32
    AF = mybir.ActivationFunctionType
    ALU = mybir.AluOpType

    nc = bacc.Bacc("TRN2", target_bir_lowering=False, debug=False)

    xT = nc.dram_tensor("xT", [D, T], bf, kind="ExternalInput")
    wq = nc.dram_tensor("wq", [D, DQ], bf, kind="ExternalInput")
    wk = nc.dram_tensor("wk", [D, DQ], bf, kind="ExternalInput")
    wv = nc.dram_tensor("wv", [D, DQ], bf, kind="ExternalInput")
    woa = nc.dram_tensor("woa", [P, D], bf, kind="ExternalInput")
    wob = nc.dram_tensor("wob", [P, D], bf, kind="ExternalInput")
    bqk = nc.dram_tensor("bqk", [P, 4], f32, kind="ExternalInput")
    bv1 = nc.dram_tensor("bv1", [1, DQ], bf, kind="ExternalInput")
    ot = nc.dram_tensor("ot", [D, T], f32, kind="ExternalOutput")

    with TileContext(nc) as tc:
        with (
            tc.tile_pool(name="per", bufs=1) as per,
            tc.tile_pool(name="xp", bufs=2) as xp,
            tc.tile_pool(name="ptp", bufs=4) as ptp,
            tc.tile_pool(name="smp", bufs=3) as smp,
            tc.tile_pool(name="obp", bufs=3) as obp,
            tc.tile_pool(name="psA", bufs=2, space="PSUM") as psA,
            tc.tile_pool(name="psS", bufs=2, space="PSUM") as psS,
            tc.tile_pool(name="psC", bufs=2, space="PSUM") as psC,
        ):
            # ---- persistent tensors ----
            qt = [
                per.tile([P, T], bf, tag=f"qt{i}", name=f"qt{i}") for i in range(2)
            ]
            kt = [
                per.tile([P, T], bf, tag=f"kt{i}", name=f"kt{i}") for i in range(2)
            ]
            ctxt = [
                per.tile([P, T], bf, tag=f"ctxt{i}", name=f"ctxt{i}")
                for i in range(2)
            ]
            vsb = per.tile([P, NTC * VST], bf, tag="vsb")
            wq_sb = per.tile([P, NDC * DQ], bf, tag="wq")
            wk_sb = per.tile([P, NDC * DQ], bf, tag="wk")
            wv_sb = per.tile([P, NDC * DQ], bf, tag="wv")
            woa_sb = per.tile([P, D], bf, tag="woa")
            wob_sb = per.tile([P, D], bf, tag="wob")
            bqk_sb = per.tile([P, 4], f32, tag="bqk")
            bv1_sb = per.tile([1, DQ], bf, tag="bv1")
            ones_sb = per.tile([1, P], bf, tag="ones")

            # ---- loads ----
            for w_sb, w_dr in ((wq_sb, wq), (wk_sb, wk), (wv_sb, wv)):
                nc.sync.dma_start(
                    w_sb[:].rearrange("p (c n) -> p c n", c=NDC),
                    w_dr[:, :].rearrange("(c p) n -> p c n", p=P),
                )
            nc.sync.dma_start(woa_sb[:], woa[:, :])
            nc.sync.dma_start(wob_sb[:], wob[:, :])
            nc.sync.dma_start(bqk_sb[:], bqk[:, :])
            nc.sync.dma_start(bv1_sb[:], bv1[:, :])
            nc.vector.memset(ones_sb[:], 1.0)
            # ones columns for the [V|1] trick (data copies overwrite the rest)
            nc.vector.memset(vsb[:], 1.0)

            # ---- phase B: Q/K/V projections ----
            for tb in range(NQB):
                xblk = xp.tile([P, NDC * TQB], bf, tag="xblk")
                nc.sync.dma_start(
                    xblk[:].rearrange("p (c t) -> p c t", c=NDC),
                    xT[:, :].rearrange("(c p) t -> p c t", p=P)[
                        :, :, tb * TQB : (tb + 1) * TQB
                    ],
                )
                for w_sb, dst, bcol in ((wq_sb, qt, 0), (wk_sb, kt, 2)):
                    for j in range(2):
                        ps = psA.tile([P, TQB], f32, tag="proj")
                        for d in range(NDC):
                            nc.tensor.matmul(
                                ps[:],
                                w_sb[:, d * DQ + j * P : d * DQ + (j + 1) * P],
                                xblk[:, d * TQB : (d + 1) * TQB],
                                start=(d == 0),
                                stop=(d == NDC - 1),
                            )
                        nc.vector.tensor_scalar_add(
                            dst[j][:, tb * TQB : (tb + 1) * TQB],
                            ps[:],
                            bqk_sb[:, bcol + j : bcol + j + 1],
                        )
                for t4 in range(4):
                    tc_ = tb * 4 + t4
                    ps = psA.tile([P, TQB], f32, tag="proj")
                    for d in range(NDC):
                        nc.tensor.matmul(
                            ps[:, :DQ],
                            xblk[:, d * TQB + t4 * P : d * TQB + (t4 + 1) * P],
                            wv_sb[:, d * DQ : (d + 1) * DQ],
                            start=(d == 0),
                            stop=False,
                        )
                    nc.tensor.matmul(
                        ps[:, :DQ], ones_sb[:, :], bv1_sb[:, :],
                        start=False, stop=True,
                    )
                    dst = vsb[:, tc_ * VST : tc_ * VST + 4 * 65].rearrange(
                        "p (h d) -> p h d", h=HPC
                    )
                    nc.vector.tensor_copy(
                        dst[:, :, 0:64],
                        ps[:, :DQ].rearrange("p (h d) -> p h d", d=HD),
                    )

            # ---- phase C: streaming causal attention ----
            for h in range(HPC):
                g2, off = h // 2, (h % 2) * 64
                for qb in range(NQB):
                    nchunks = 4 * (qb + 1)
                    psc = psC.tile([P, TQB], f32, tag="ctx")
                    for gp in range(nchunks // 2):
                        pss = psS.tile([P, 1024], f32, tag="scores")
                        pt = ptp.tile([P, 1024], bf, tag="pt")
                        for jj in range(2):
                            tk0 = (gp * 2 + jj) * P
                            nc.tensor.matmul(
                                pss[:, jj * 512 : (jj + 1) * 512],
                                kt[g2][off : off + 64, tk0 : tk0 + P],
                                qt[g2][off : off + 64, qb * TQB : (qb + 1) * TQB],
                                start=True,
                                stop=True,
                            )
                        nc.scalar.activation(
                            pt[:], pss[:], AF.Exp, scale=1.0 / math.sqrt(HD)
                        )
                        for jj in range(2):
                            tk0 = (gp * 2 + jj) * P
                            if tk0 >= qb * TQB:
                                # keep where tk0+p <= qb*TQB+f, i.e.
                                # (-1)*p + 1*f + (qb*TQB - tk0) >= 0
                                nc.gpsimd.affine_select(
                                    pt[:, jj * 512 : (jj + 1) * 512],
                                    pt[:, jj * 512 : (jj + 1) * 512],
                                    pattern=[[1, 512]],
                                    compare_op=ALU.is_ge,
                                    fill=0.0,
                                    base=qb * TQB - tk0,
                                    channel_multiplier=-1,
                                )
                        for jj in range(2):
                            ck = gp * 2 + jj
                            nc.tensor.matmul(
                                psc[:],
                                vsb[:, ck * VST + h * 65 : ck * VST + h * 65 + P],
                                pt[:, jj * 512 : (jj + 1) * 512],
                                start=(ck == 0),
                                stop=(ck == nchunks - 1),
                            )
                    # normalize: rows 0-63 ctx^T, row 64 = l
                    lsb = smp.tile([1, TQB], f32, tag="l")
                    nc.vector.tensor_copy(lsb[:], psc[64:65, :])
                    lb = smp.tile([64, TQB], f32, tag="lb")
                    nc.gpsimd.partition_broadcast(lb[:], lsb[:])
                    rb = smp.tile([64, TQB], f32, tag="rb")
                    nc.vector.reciprocal(rb[:], lb[:])
                    qs = slice(qb * TQB, (qb + 1) * TQB)
                    if off == 0:
                        nc.vector.tensor_mul(
                            ctxt[g2][0:64, qs], psc[0:64, :], rb[:]
                        )
                    else:
                        ctmp = smp.tile([64, TQB], bf, tag="ctmp")
                        nc.vector.tensor_mul(ctmp[:], psc[0:64, :], rb[:])
                        nc.sync.dma_start(ctxt[g2][64:128, qs], ctmp[:])

            # ---- phase D: partial output projection (transposed) ----
            for qb in range(NQB):
                qs = slice(qb * TQB, (qb + 1) * TQB)
                for n in range(NDC):
                    pso = psA.tile([P, TQB], f32, tag="proj")
                    nc.tensor.matmul(
                        pso[:], woa_sb[:, n * P : (n + 1) * P], ctxt[0][:, qs],
                        start=True, stop=False,
                    )
                    nc.tensor.matmul(
                        pso[:], wob_sb[:, n * P : (n + 1) * P], ctxt[1][:, qs],
                        start=False, stop=True,
                    )
                    osb = obp.tile([P, TQB], f32, tag="osb")
                    nc.vector.tensor_copy(osb[:], pso[:])
                    nc.sync.dma_start(ot[n * P : (n + 1) * P, qs], osb[:])

    nc.compile()
    return nc


def _get_nc():
    if "nc" not in _NC_CACHE:
        _NC_CACHE["nc"] = _build_nc()
    return _NC_CACHE["nc"]


def _in_maps(x, Wq, bq, Wk, bk, Wv, bv, Wo, bo):
    import ml_dtypes

    bf = ml_dtypes.bfloat16
    maps = []
    for c in range(NCORES):
        b, hg = divmod(c, 4)
        cs = slice(hg * DQ, (hg + 1) * DQ)
        bqk_pack = np.stack(
            [
                bq[cs][0:128], bq[cs][128:256],
                bk[cs][0:128], bk[cs][128:256],
            ],
            axis=1,
        ).astype(np.float32)
        maps.append(
            {
                "xT": np.ascontiguousarray(x[b].T).astype(bf),
                "wq": Wq[:, cs].astype(bf),
                "wk": Wk[:, cs].astype(bf),
                "wv": Wv[:, cs].astype(bf),
                "woa": Wo[cs, :][0:128].astype(bf),
                "wob": Wo[cs, :][128:256].astype(bf),
                "bqk": np.ascontiguousarray(bqk_pack),
                "bv1": bv[cs].reshape(1, DQ).astype(bf),
            }
        )
    return maps


def kernel(x, Wq, bq, Wk, bk, Wv, bv, Wo, bo):
    from concourse.bass_utils import run_bass_kernel_spmd

    nc = _get_nc()
    maps = _in_maps(x, Wq, bq, Wk, bk, Wv, bv, Wo, bo)
    res = run_bass_kernel_spmd(nc, maps, list(range(NCORES)))
    out = np.zeros((B, T, D), np.float32)
    for b in range(B):
        acc = res.results[b * 4]["ot"].astype(np.float32)
        for g in range(1, 4):
            acc = acc + res.results[b * 4 + g]["ot"]
        out[b] = acc.T + bo.astype(np.float32)
    return out
